# revision 41
# baseline (speedup 1.0000x reference)
"""CQT layer kernel for Trainium2 (8 NeuronCores, SPMD) — sparsity-aware.

The strided conv (hop 128 == PE contraction tile) is a chunked matmul:
  out[c, b, t] = sum_k  W[c, 128k:128k+128] . xT_b[:, t+k]
where xT_b is the zero-padded audio reshaped to [128, 672] (a free reshape,
because hop == 128).

The CQT filterbank rows are hann-windowed exponentials whose support Nk
decays geometrically with bin index — only ~18% of the dense [1056, 63872]
weight array is nonzero, in centered, nested intervals.  We detect the
nonzero chunk range at runtime per group of 64 bins (re+im paired -> 128
channels, supports sorted so group cost = widest member) and only stream /
multiply those chunks.  Each group's chunk range is split 8 ways across the
cores (contraction-parallel); every core runs the identical program on its
own shifted x-window + weight slice, and the host sums the per-core partial
convolutions (the "all-reduce" of the hint, done host-side on 1.6 MB).

Segments execute smallest-first so the per-segment PSUM drains (vector copy
+ out-DMA on the scalar HWDGE queue) overlap the big segments' matmuls and
only the largest segment's drain trails the final matmul.

Magnitude + power_to_db run on host, with an exact fp64 recompute of the few
near-silent bins where reduced-precision matmul error would be audible in dB.

Self-contained: only needs numpy + the concourse toolchain at /opt/trn_rl_repo.
"""
import os
import sys

sys.path.insert(0, "/opt/trn_rl_repo")
import numpy as np

# ---- problem constants (hardcoded from the CQT layer spec) ----
B = 2
AUDIO_LEN = 22016
N_BINS = 528
NCH = 2 * N_BINS          # 1056 conv channels (re, im)
HOP = 128
FRAMES = 173
AMIN = 1e-10
TOP_DB = 80.0

K = 128                   # PE contraction tile == HOP
NCHUNK = 499              # ceil(L / 128); holds for L in [63745, 63872]
LPAD = NCHUNK * K         # 63872
NT = 174                  # frames padded to even
NROW = NCHUNK + NT - 1    # 672 columns of xT per batch
N_CORES = 8
GBINS = 64                # bins per channel group (re+im pair -> 128 channels)
NSEG = (N_BINS + GBINS - 1) // GBINS   # 9 groups

DTYPE = os.environ.get("CQT_DTYPE", "float16")  # float16 | float32r
# device matmul relative error (vs conv rms); drives the host refinement
# threshold for near-silent bins.  abs_err ~= eps * rms(conv) because the
# per-product rounding errors accumulate like the products themselves.
_CONV_EPS = {"float16": 1.2e-3, "float32r": 5e-4, "bfloat16": 5e-3}
DB_ERR_TARGET = 0.02      # refine bins whose worst-case dB error exceeds this

_prog_cache = {}


def _np_cast(a):
    if DTYPE == "float16":
        return a.astype(np.float16)
    if DTYPE == "bfloat16":
        import ml_dtypes
        return a.astype(ml_dtypes.bfloat16)
    return a  # float32r: raw fp32 bits


def _build_program(jsegs):
    """jsegs: per-execution-position unit counts (ascending)."""
    from concourse import bacc, mybir
    from concourse.tile import TileContext

    dt = mybir.dt
    DT = getattr(dt, DTYPE)

    jsegs = list(jsegs)
    nseg = len(jsegs)
    U = sum(jsegs)                                   # matmul units per core
    xwidths = [(j + NT - 1) * B for j in jsegs]      # x-window cols per segment
    XWCOLS = sum(xwidths)
    xoffs = np.concatenate([[0], np.cumsum(xwidths)]).astype(int)
    uoffs = np.concatenate([[0], np.cumsum(jsegs)]).astype(int)

    nc = bacc.Bacc(None, target_bir_lowering=False)
    xw_p = nc.declare_dram_parameter("xw", [K, XWCOLS], DT, isOutput=False)
    wm_p = nc.declare_dram_parameter("wm", [K, U * K], DT, isOutput=False)
    # outputs drained as fp16: halves the out-DMA bytes; the fp16 rounding
    # (~5e-4 relative) is below the fp16-matmul accumulation error
    om_p = nc.declare_dram_parameter("om", [K, nseg * B * NT], dt.float16,
                                     isOutput=True)

    # weight DMA groups: small first so the PE starts streaming early,
    # then steady-state ~1MB DMAs
    GROUP = int(os.environ.get("CQT_GROUP", "12"))
    wgroups = []
    u0 = 0
    ramp = [int(v) for v in os.environ.get("CQT_RAMP", "2,4,8,16").split(",") if v]
    for g in ramp:
        if u0 >= U:
            break
        wgroups.append((u0, min(g, U - u0)))
        u0 += wgroups[-1][1]
    while u0 < U:
        cnt = min(GROUP, U - u0)
        wgroups.append((u0, cnt))
        u0 += cnt
    N_WARM = int(os.environ.get("CQT_WARM", "8"))   # HAM warm-up matmuls
    # contiguous runs of small slots share one batched out-DMA: for each
    # position, the inclusive start of the small-run it terminates (or None)
    drain_from = []
    run_start = None
    for p, j in enumerate(jsegs):
        if j >= 8:
            drain_from.append(p)
            run_start = None
        else:
            if run_start is None:
                run_start = p
            last_of_run = p + 1 >= nseg or jsegs[p + 1] >= 8
            drain_from.append(run_start if last_of_run else None)

    with TileContext(nc) as tc:
        with (
            tc.tile_pool(name="stat", bufs=1) as stat,
            tc.tile_pool(name="opool", bufs=1) as opool,
            tc.tile_pool(name="psw", bufs=1, space="PSUM") as psw,
            tc.tile_pool(name="ps", bufs=4, space="PSUM") as ps,
        ):
            # PE warm-up on a memset tile: no DMA dependency, so it starts
            # right after the framework preamble (DVE memset is ~300ns) and
            # keeps PE busy through the input-DMA window — HAM reaches
            # 2.4 GHz before real work
            warm_sb = stat.tile([K, B * NT], DT)
            nc.vector.memset(warm_sb[:], 0.0)
            ps_warm = psw.tile([K, B * NT], dt.float32)
            for _ in range(N_WARM):
                nc.tensor.matmul(ps_warm[:], warm_sb[:, :K], warm_sb[:],
                                 start=True, stop=True)

            xw_sb = stat.tile([K, XWCOLS], DT)
            wm_sb = stat.tile([K, U * K], DT)
            om_sb = opool.tile([K, nseg * B * NT], dt.float16)

            # input DMA issue: the first (biggest) slot's x window is the
            # only x on the critical path — it goes on the scalar HWDGE
            # queue; weight groups stream on the sync queue with the
            # remaining x windows inserted mid-ramp so their transfers
            # don't steal early DMA-engine time.
            x1 = int(xoffs[1])
            nc.scalar.dma_start(xw_sb[:, :x1], xw_p[:, :x1])
            xr_after = min(len(ramp) + 1, len(wgroups) - 1)
            for gi, (w0, cnt) in enumerate(wgroups):
                # ramp groups 1-2 issue from the scalar queue in parallel
                # with the sync queue so the supply front builds faster
                eng = nc.scalar if gi in (1, 2) else nc.sync
                eng.dma_start(wm_sb[:, w0 * K:(w0 + cnt) * K],
                              wm_p[:, w0 * K:(w0 + cnt) * K])
                if gi == xr_after and XWCOLS > x1:
                    nc.sync.dma_start(xw_sb[:, x1:], xw_p[:, x1:])

            half = B * NT // 2
            for g in range(nseg):
                J = jsegs[g]
                x3 = xw_sb[:, xoffs[g]:xoffs[g + 1]].rearrange(
                    "p (t b) -> p t b", b=B)
                ps_g = ps.tile([K, B * NT], dt.float32, tag="seg")
                pg3 = ps_g[:].rearrange("p (t b) -> p t b", b=B)
                for j in range(J):
                    u = uoffs[g] + j
                    nc.tensor.matmul(pg3, wm_sb[:, u * K:(u + 1) * K],
                                     x3[:, j:j + NT, :],
                                     start=(j == 0), stop=(j == J - 1))
                # drain: fp32 PSUM -> fp16 SBUF copy, then out-DMA on the
                # scalar HWDGE queue; runs of small slots share one DMA.
                # The final slot is the kernel tail: its halves drain on
                # disjoint engine pairs (vector+scalar, scalar+sync).
                if g == nseg - 1 and J >= 8:
                    for h in range(2):
                        sl = slice(g * B * NT + h * half,
                                   g * B * NT + (h + 1) * half)
                        pl = slice(h * half, (h + 1) * half)
                        if h == 1:
                            nc.scalar.copy(om_sb[:, sl], ps_g[:, pl])
                            nc.sync.dma_start(om_p[:, sl], om_sb[:, sl])
                        else:
                            nc.vector.tensor_copy(om_sb[:, sl], ps_g[:, pl])
                            nc.scalar.dma_start(om_p[:, sl], om_sb[:, sl])
                else:
                    sl = slice(g * B * NT, (g + 1) * B * NT)
                    nc.vector.tensor_copy(om_sb[:, sl], ps_g[:])
                    if drain_from[g] is not None:
                        dsl = slice(drain_from[g] * B * NT, (g + 1) * B * NT)
                        nc.scalar.dma_start(om_p[:, dsl], om_sb[:, dsl])

    nc.finalize()
    return nc


LAST_RESULTS = None

SLOT_LADDER = [64, 32, 16, 8, 4, 2, 1, 1]   # per-core PSUM segment capacities


def _group_channels(g):
    """Row indices in the [1056, L] weight matrix for group g."""
    b0, b1 = GBINS * g, min(GBINS * (g + 1), N_BINS)
    return np.r_[b0:b1, N_BINS + b0:N_BINS + b1]


def _pack_slots(lens, glo):
    """Pack each group's chunk range into the global pool of fixed-size
    slots (8 cores x SLOT_LADDER), splitting ranges into contiguous pieces.
    Returns (jsegs, assign) with assign[core][pos] = (group, k0, span),
    or None if the ladder can't cover the work."""
    sizes = sorted(set(SLOT_LADDER), reverse=True)
    avail = {s: N_CORES * SLOT_LADDER.count(s) for s in sizes}
    pieces = {s: [] for s in sizes}
    for g in sorted(range(len(lens)), key=lambda g: -lens[g]):
        rem, k = lens[g], glo[g]
        for s in sizes:
            while avail[s] and rem >= s:
                avail[s] -= 1
                pieces[s].append((g, k, s))
                k += s
                rem -= s
        if rem > 0:
            cand = [s for s in sizes if s >= rem and avail[s]]
            if not cand:
                return None
            s = min(cand)
            avail[s] -= 1
            pieces[s].append((g, k, rem))
    # execution: biggest slot first (its x window is the only one on the
    # critical path at start), then the tiny slots (their serialized drain
    # copies overlap the mid slots' matmuls), mid slots ascending so the
    # kernel tail is a single copy + DMA for the final slot
    desc = sorted(SLOT_LADDER, reverse=True)
    jsegs = desc[:1] + [j for j in desc[1:] if j < 8] + \
        sorted(j for j in desc[1:] if j >= 8)
    assign = [[None] * len(jsegs) for _ in range(N_CORES)]
    for p, s in enumerate(jsegs):
        for i in range(N_CORES):
            if pieces[s]:
                assign[i][p] = pieces[s].pop()
    return jsegs, assign


def kernel(y, kern_r, kern_i):
    global LAST_RESULTS
    from concourse.bass_utils import run_bass_kernel_spmd

    y = np.asarray(y, dtype=np.float32)
    kern_r = np.asarray(kern_r, dtype=np.float32)
    kern_i = np.asarray(kern_i, dtype=np.float32)

    # ---- host prep: weights -> chunked layout, detect nonzero chunk ranges ----
    L_in = kern_r.shape[1]                                 # 63864 from the layer
    pad = L_in // 2
    assert (NCHUNK - 1) * K < L_in <= LPAD, L_in
    W = np.concatenate([kern_r, kern_i], axis=0)          # [1056, L]
    Wp = np.zeros((NCH, LPAD), np.float32)
    Wp[:, :L_in] = W
    Wk = Wp.reshape(NCH, NCHUNK, K)                        # [c, k, l]

    occ = np.abs(Wk).sum(axis=2) > 0                       # [c, k] chunk occupancy
    glo, glen = [], []
    for g in range(NSEG):
        occ_g = occ[_group_channels(g)].any(axis=0)
        nz = np.flatnonzero(occ_g)
        lo, hi = (int(nz[0]), int(nz[-1]) + 1) if len(nz) else (0, 1)
        glo.append(lo)
        glen.append(hi - lo)

    packed = _pack_slots(glen, glo)
    if packed is not None:
        jsegs, assign = packed
    else:
        # fallback: split every group's range 8 ways (one slot per group)
        jlen = [int(np.ceil(l / N_CORES)) for l in glen]
        order = sorted(range(NSEG), key=lambda g: -jlen[g])
        jsegs = [jlen[g] for g in order]
        assign = [
            [(g, glo[g] + i * jlen[g], max(0, min(jlen[g], glen[g] - i * jlen[g])))
             for g in order]
            for i in range(N_CORES)
        ]

    key = tuple(jsegs)
    if key not in _prog_cache:
        _prog_cache[key] = _build_program(jsegs)
    nc = _prog_cache[key]
    nseg = len(jsegs)

    U = sum(jsegs)
    xwidths = [(j + NT - 1) * B for j in jsegs]
    xoffs = np.concatenate([[0], np.cumsum(xwidths)]).astype(int)
    uoffs = np.concatenate([[0], np.cumsum(jsegs)]).astype(int)

    # ---- host prep: audio -> xT [B, 128, 672] ----
    x_pad = np.zeros((B, NROW * K), np.float32)
    x_pad[:, pad:pad + AUDIO_LEN] = y
    xT = np.ascontiguousarray(x_pad.reshape(B, NROW, K).transpose(0, 2, 1))

    in_maps = []
    for i in range(N_CORES):
        xw = np.zeros((K, int(xoffs[-1])), np.float32)
        wm = np.zeros((K, U * K), np.float32)
        for p in range(nseg):
            if assign[i][p] is None:
                continue
            g, k0, span = assign[i][p]
            J = jsegs[p]
            # x window: xT columns [k0, k0 + J + NT - 1), (t, b)-interleaved
            w_cols = J + NT - 1
            lo_c = max(0, k0)
            hi_c = min(NROW, k0 + w_cols)
            if hi_c > lo_c:
                win = xT[:, :, lo_c:hi_c]                  # [B, K, n]
                seg = xw[:, xoffs[p]:xoffs[p + 1]].reshape(K, w_cols, B)
                seg[:, lo_c - k0:hi_c - k0, :] = win.transpose(1, 2, 0)
            # weights: chunks [k0, k0 + span), channels of group g (zero-pad)
            ch = _group_channels(g)
            k1 = min(k0 + span, NCHUNK)
            if k1 > k0:
                blk = Wk[ch, k0:k1, :]                     # [nch, n, K]
                dst = wm[:, uoffs[p] * K:(uoffs[p] + (k1 - k0)) * K]
                dst.reshape(K, k1 - k0, K)[:, :, :len(ch)] = blk.transpose(2, 1, 0)
        in_maps.append({"xw": _np_cast(xw), "wm": _np_cast(wm)})

    LAST_RESULTS = run_bass_kernel_spmd(
        nc, in_maps, list(range(N_CORES)),
        trace=bool(os.environ.get("CQT_TRACE")),
    )
    results = LAST_RESULTS.results

    # ---- host post: sum per-core partials, assemble conv ----
    conv = np.zeros((NCH, B, FRAMES), np.float64)
    for i in range(N_CORES):
        om = results[i]["om"].reshape(K, nseg, NT, B).astype(np.float64)
        for p in range(nseg):
            if assign[i][p] is None:
                continue
            ch = _group_channels(assign[i][p][0])
            conv[ch] += om[:len(ch), p, :FRAMES, :].transpose(0, 2, 1)

    re = conv[:N_BINS]                                     # [528, B, 173]
    im = conv[N_BINS:]
    mag = np.sqrt(re * re + im * im)                       # [528, B, 173]

    # ---- host refinement: exact recompute of near-silent bins ----
    conv_rms = float(np.sqrt(np.mean(mag * mag)))
    err_abs = _CONV_EPS.get(DTYPE, 1e-3) * conv_rms
    thresh = 4.343 * err_abs / DB_ERR_TARGET
    fix = np.argwhere(mag < thresh)                        # rows: (bin, b, t)
    if len(fix):
        W64 = W.astype(np.float64)
        xp64 = x_pad.astype(np.float64)
        for b in range(B):
            sel = fix[fix[:, 1] == b]
            if not len(sel):
                continue
            for t in np.unique(sel[:, 2]):
                bins = sel[sel[:, 2] == t][:, 0]
                win = xp64[b, t * HOP:t * HOP + L_in]
                re[bins, b, t] = W64[bins] @ win
                im[bins, b, t] = W64[bins + N_BINS] @ win
        mag = np.sqrt(re * re + im * im)

    ref = max(mag.max(), AMIN)
    log_spec = 10.0 * np.log10(np.maximum(mag, AMIN)) - 10.0 * np.log10(ref)
    log_spec = np.maximum(log_spec, log_spec.max() - TOP_DB)
    return np.ascontiguousarray(log_spec.transpose(1, 2, 0)).astype(np.float32)


# revision 44
# speedup vs baseline: 1.0557x; 1.0557x over previous
"""CQT layer kernel for Trainium2 (8 NeuronCores, SPMD) — sparsity-aware.

The strided conv (hop 128 == PE contraction tile) is a chunked matmul:
  out[c, b, t] = sum_k  W[c, 128k:128k+128] . xT_b[:, t+k]
where xT_b is the zero-padded audio reshaped to [128, 672] (a free reshape,
because hop == 128).

The CQT filterbank rows are hann-windowed exponentials whose support Nk
decays geometrically with bin index — only ~18% of the dense [1056, 63872]
weight array is nonzero, in centered, nested intervals.  We detect the
nonzero chunk range at runtime per group of 64 bins (re+im paired -> 128
channels, supports sorted so group cost = widest member) and only stream /
multiply those chunks.  Each group's chunk range is split 8 ways across the
cores (contraction-parallel); every core runs the identical program on its
own shifted x-window + weight slice, and the host sums the per-core partial
convolutions (the "all-reduce" of the hint, done host-side on 1.6 MB).

Per-core work is packed into fixed PSUM-segment slots (64/32/16/8/4/2/1/1
chunks) by an exact subset-sum assignment — 128 matmul units per core vs
1023 total.  The biggest slot executes first (only its 121KB x window is on
the critical path at start), the tiny slots next (their drains overlap the
mid slots' matmuls), and the kernel tail is a single segment drain.

Magnitude + power_to_db run on host, with an exact fp64 recompute of the few
near-silent bins where reduced-precision matmul error would be audible in dB.

Self-contained: only needs numpy + the concourse toolchain at /opt/trn_rl_repo.
"""
import os
import sys

sys.path.insert(0, "/opt/trn_rl_repo")
import numpy as np

# ---- problem constants (hardcoded from the CQT layer spec) ----
B = 2
AUDIO_LEN = 22016
N_BINS = 528
NCH = 2 * N_BINS          # 1056 conv channels (re, im)
HOP = 128
FRAMES = 173
AMIN = 1e-10
TOP_DB = 80.0

K = 128                   # PE contraction tile == HOP
NCHUNK = 499              # ceil(L / 128); holds for L in [63745, 63872]
LPAD = NCHUNK * K         # 63872
NT = 174                  # frames padded to even
NROW = NCHUNK + NT - 1    # 672 columns of xT per batch
N_CORES = 8
GBINS = 64                # bins per channel group (re+im pair -> 128 channels)
NSEG = (N_BINS + GBINS - 1) // GBINS   # 9 groups

DTYPE = os.environ.get("CQT_DTYPE", "float16")  # float16 | float32r
# device matmul relative error (vs conv rms); drives the host refinement
# threshold for near-silent bins.  abs_err ~= eps * rms(conv) because the
# per-product rounding errors accumulate like the products themselves.
_CONV_EPS = {"float16": 1.2e-3, "float32r": 5e-4, "bfloat16": 5e-3}
DB_ERR_TARGET = 0.02      # refine bins whose worst-case dB error exceeds this

_prog_cache = {}


def _np_cast(a):
    if DTYPE == "float16":
        return a.astype(np.float16)
    if DTYPE == "bfloat16":
        import ml_dtypes
        return a.astype(ml_dtypes.bfloat16)
    return a  # float32r: raw fp32 bits


def _build_program(jsegs):
    """jsegs: per-execution-position unit counts."""
    from concourse import bacc, mybir
    from concourse.tile import TileContext

    dt = mybir.dt
    DT = getattr(dt, DTYPE)

    jsegs = list(jsegs)
    nseg = len(jsegs)
    U = sum(jsegs)                                   # matmul units per core
    xwidths = [(j + NT - 1) * B for j in jsegs]      # x-window cols per segment
    XWCOLS = sum(xwidths)
    xoffs = np.concatenate([[0], np.cumsum(xwidths)]).astype(int)
    uoffs = np.concatenate([[0], np.cumsum(jsegs)]).astype(int)

    nc = bacc.Bacc(None, target_bir_lowering=False)
    xw_p = nc.declare_dram_parameter("xw", [K, XWCOLS], DT, isOutput=False)
    wm_p = nc.declare_dram_parameter("wm", [K, U * K], DT, isOutput=False)
    # outputs drained as fp16: halves the out-DMA bytes; the fp16 rounding
    # (~5e-4 relative) is below the fp16-matmul accumulation error
    om_p = nc.declare_dram_parameter("om", [K, nseg * B * NT], dt.float16,
                                     isOutput=True)

    # weight DMA groups: small first so the PE starts streaming early,
    # then steady-state ~1MB DMAs
    GROUP = int(os.environ.get("CQT_GROUP", "12"))
    wgroups = []
    u0 = 0
    ramp = [int(v) for v in os.environ.get("CQT_RAMP", "2,4,8,16").split(",") if v]
    for g in ramp:
        if u0 >= U:
            break
        wgroups.append((u0, min(g, U - u0)))
        u0 += wgroups[-1][1]
    while u0 < U:
        cnt = min(GROUP, U - u0)
        wgroups.append((u0, cnt))
        u0 += cnt
    N_WARM = int(os.environ.get("CQT_WARM", "8"))   # HAM warm-up matmuls
    # contiguous runs of small slots share one batched out-DMA: for each
    # position, the inclusive start of the small-run it terminates (or None)
    drain_from = []
    run_start = None
    for p, j in enumerate(jsegs):
        if j >= 8:
            drain_from.append(p)
            run_start = None
        else:
            if run_start is None:
                run_start = p
            last_of_run = p + 1 >= nseg or jsegs[p + 1] >= 8
            drain_from.append(run_start if last_of_run else None)

    with TileContext(nc) as tc:
        with (
            tc.tile_pool(name="stat", bufs=1) as stat,
            tc.tile_pool(name="opool", bufs=1) as opool,
            tc.tile_pool(name="psw", bufs=1, space="PSUM") as psw,
            tc.tile_pool(name="ps", bufs=4, space="PSUM") as ps,
        ):
            # PE warm-up on a memset tile: no DMA dependency, so it starts
            # right after the framework preamble (DVE memset is ~300ns) and
            # keeps PE busy through the input-DMA window — HAM reaches
            # 2.4 GHz before real work
            warm_sb = stat.tile([K, B * NT], DT)
            nc.vector.memset(warm_sb[:], 0.0)
            ps_warm = psw.tile([K, B * NT], dt.float32)
            for _ in range(N_WARM):
                nc.tensor.matmul(ps_warm[:], warm_sb[:, :K], warm_sb[:],
                                 start=True, stop=True)

            xw_sb = stat.tile([K, XWCOLS], DT)
            wm_sb = stat.tile([K, U * K], DT)
            om_sb = opool.tile([K, nseg * B * NT], dt.float16)

            # input DMA issue: the first (biggest) slot's x window is the
            # only x on the critical path — it goes on the scalar HWDGE
            # queue; weight groups stream on the sync queue with the
            # remaining x windows inserted mid-ramp so their transfers
            # don't steal early DMA-engine time.
            x1 = int(xoffs[1])
            nc.scalar.dma_start(xw_sb[:, :x1], xw_p[:, :x1])
            xr_after = min(len(ramp) + 1, len(wgroups) - 1)
            for gi, (w0, cnt) in enumerate(wgroups):
                nc.sync.dma_start(wm_sb[:, w0 * K:(w0 + cnt) * K],
                                  wm_p[:, w0 * K:(w0 + cnt) * K])
                if gi == xr_after and XWCOLS > x1:
                    nc.sync.dma_start(xw_sb[:, x1:], xw_p[:, x1:])

            half = B * NT // 2
            for g in range(nseg):
                J = jsegs[g]
                x3 = xw_sb[:, xoffs[g]:xoffs[g + 1]].rearrange(
                    "p (t b) -> p t b", b=B)
                ps_g = ps.tile([K, B * NT], dt.float32, tag="seg")
                pg3 = ps_g[:].rearrange("p (t b) -> p t b", b=B)
                for j in range(J):
                    u = uoffs[g] + j
                    nc.tensor.matmul(pg3, wm_sb[:, u * K:(u + 1) * K],
                                     x3[:, j:j + NT, :],
                                     start=(j == 0), stop=(j == J - 1))
                # drain: fp32 PSUM -> fp16 SBUF copy, then out-DMA on the
                # scalar HWDGE queue; runs of small slots share one DMA.
                # The final slot is the kernel tail: its halves drain on
                # disjoint engine pairs (vector+scalar, scalar+sync).
                if g == nseg - 1 and J >= 8:
                    for h in range(2):
                        sl = slice(g * B * NT + h * half,
                                   g * B * NT + (h + 1) * half)
                        pl = slice(h * half, (h + 1) * half)
                        if h == 1:
                            nc.scalar.copy(om_sb[:, sl], ps_g[:, pl])
                            nc.sync.dma_start(om_p[:, sl], om_sb[:, sl])
                        else:
                            nc.vector.tensor_copy(om_sb[:, sl], ps_g[:, pl])
                            nc.scalar.dma_start(om_p[:, sl], om_sb[:, sl])
                else:
                    sl = slice(g * B * NT, (g + 1) * B * NT)
                    nc.vector.tensor_copy(om_sb[:, sl], ps_g[:])
                    if drain_from[g] is not None:
                        dsl = slice(drain_from[g] * B * NT, (g + 1) * B * NT)
                        nc.scalar.dma_start(om_p[:, dsl], om_sb[:, dsl])

    nc.finalize()
    return nc


LAST_RESULTS = None

SLOT_LADDER = [64, 32, 16, 8, 4, 2, 1, 1]   # per-core PSUM segment capacities


def _group_channels(g):
    """Row indices in the [1056, L] weight matrix for group g."""
    b0, b1 = GBINS * g, min(GBINS * (g + 1), N_BINS)
    return np.r_[b0:b1, N_BINS + b0:N_BINS + b1]


def _pack_slots(lens, glo):
    """Pack each group's chunk range into the global pool of fixed-size
    slots (8 cores x SLOT_LADDER), splitting ranges into contiguous pieces.
    Returns (jsegs, assign) with assign[core][pos] = (group, k0, span),
    or None if the ladder can't cover the work."""
    sizes = sorted(set(SLOT_LADDER), reverse=True)
    avail = {s: N_CORES * SLOT_LADDER.count(s) for s in sizes}
    pieces = {s: [] for s in sizes}
    for g in sorted(range(len(lens)), key=lambda g: -lens[g]):
        rem, k = lens[g], glo[g]
        for s in sizes:
            while avail[s] and rem >= s:
                avail[s] -= 1
                pieces[s].append((g, k, s))
                k += s
                rem -= s
        if rem > 0:
            cand = [s for s in sizes if s >= rem and avail[s]]
            if not cand:
                return None
            s = min(cand)
            avail[s] -= 1
            pieces[s].append((g, k, rem))
    # execution: biggest slot first (its x window is the only one on the
    # critical path at start), then the tiny slots (their serialized drain
    # copies overlap the mid slots' matmuls), mid slots ascending so the
    # kernel tail is a single copy + DMA for the final slot
    desc = sorted(SLOT_LADDER, reverse=True)
    jsegs = desc[:1] + [j for j in desc[1:] if j < 8] + \
        sorted(j for j in desc[1:] if j >= 8)
    assign = [[None] * len(jsegs) for _ in range(N_CORES)]
    for p, s in enumerate(jsegs):
        for i in range(N_CORES):
            if pieces[s]:
                assign[i][p] = pieces[s].pop()
    return jsegs, assign


def kernel(y, kern_r, kern_i):
    global LAST_RESULTS
    from concourse.bass_utils import run_bass_kernel_spmd

    y = np.asarray(y, dtype=np.float32)
    kern_r = np.asarray(kern_r, dtype=np.float32)
    kern_i = np.asarray(kern_i, dtype=np.float32)

    # ---- host prep: weights -> chunked layout, detect nonzero chunk ranges ----
    L_in = kern_r.shape[1]                                 # 63864 from the layer
    pad = L_in // 2
    assert (NCHUNK - 1) * K < L_in <= LPAD, L_in
    W = np.concatenate([kern_r, kern_i], axis=0)          # [1056, L]
    Wp = np.zeros((NCH, LPAD), np.float32)
    Wp[:, :L_in] = W
    Wk = Wp.reshape(NCH, NCHUNK, K)                        # [c, k, l]

    occ = np.abs(Wk).sum(axis=2) > 0                       # [c, k] chunk occupancy
    glo, glen = [], []
    for g in range(NSEG):
        occ_g = occ[_group_channels(g)].any(axis=0)
        nz = np.flatnonzero(occ_g)
        lo, hi = (int(nz[0]), int(nz[-1]) + 1) if len(nz) else (0, 1)
        glo.append(lo)
        glen.append(hi - lo)

    packed = _pack_slots(glen, glo)
    if packed is not None:
        jsegs, assign = packed
    else:
        # fallback: split every group's range 8 ways (one slot per group)
        jlen = [int(np.ceil(l / N_CORES)) for l in glen]
        order = sorted(range(NSEG), key=lambda g: -jlen[g])
        jsegs = [jlen[g] for g in order]
        assign = [
            [(g, glo[g] + i * jlen[g], max(0, min(jlen[g], glen[g] - i * jlen[g])))
             for g in order]
            for i in range(N_CORES)
        ]

    key = tuple(jsegs)
    if key not in _prog_cache:
        _prog_cache[key] = _build_program(jsegs)
    nc = _prog_cache[key]
    nseg = len(jsegs)

    U = sum(jsegs)
    xwidths = [(j + NT - 1) * B for j in jsegs]
    xoffs = np.concatenate([[0], np.cumsum(xwidths)]).astype(int)
    uoffs = np.concatenate([[0], np.cumsum(jsegs)]).astype(int)

    # ---- host prep: audio -> xT [B, 128, 672] ----
    x_pad = np.zeros((B, NROW * K), np.float32)
    x_pad[:, pad:pad + AUDIO_LEN] = y
    xT = np.ascontiguousarray(x_pad.reshape(B, NROW, K).transpose(0, 2, 1))

    in_maps = []
    for i in range(N_CORES):
        xw = np.zeros((K, int(xoffs[-1])), np.float32)
        wm = np.zeros((K, U * K), np.float32)
        for p in range(nseg):
            if assign[i][p] is None:
                continue
            g, k0, span = assign[i][p]
            J = jsegs[p]
            # x window: xT columns [k0, k0 + J + NT - 1), (t, b)-interleaved
            w_cols = J + NT - 1
            lo_c = max(0, k0)
            hi_c = min(NROW, k0 + w_cols)
            if hi_c > lo_c:
                win = xT[:, :, lo_c:hi_c]                  # [B, K, n]
                seg = xw[:, xoffs[p]:xoffs[p + 1]].reshape(K, w_cols, B)
                seg[:, lo_c - k0:hi_c - k0, :] = win.transpose(1, 2, 0)
            # weights: chunks [k0, k0 + span), channels of group g (zero-pad)
            ch = _group_channels(g)
            k1 = min(k0 + span, NCHUNK)
            if k1 > k0:
                blk = Wk[ch, k0:k1, :]                     # [nch, n, K]
                dst = wm[:, uoffs[p] * K:(uoffs[p] + (k1 - k0)) * K]
                dst.reshape(K, k1 - k0, K)[:, :, :len(ch)] = blk.transpose(2, 1, 0)
        in_maps.append({"xw": _np_cast(xw), "wm": _np_cast(wm)})

    LAST_RESULTS = run_bass_kernel_spmd(
        nc, in_maps, list(range(N_CORES)),
        trace=bool(os.environ.get("CQT_TRACE")),
    )
    results = LAST_RESULTS.results

    # ---- host post: sum per-core partials, assemble conv ----
    conv = np.zeros((NCH, B, FRAMES), np.float64)
    for i in range(N_CORES):
        om = results[i]["om"].reshape(K, nseg, NT, B).astype(np.float64)
        for p in range(nseg):
            if assign[i][p] is None:
                continue
            ch = _group_channels(assign[i][p][0])
            conv[ch] += om[:len(ch), p, :FRAMES, :].transpose(0, 2, 1)

    re = conv[:N_BINS]                                     # [528, B, 173]
    im = conv[N_BINS:]
    mag = np.sqrt(re * re + im * im)                       # [528, B, 173]

    # ---- host refinement: exact recompute of near-silent bins ----
    conv_rms = float(np.sqrt(np.mean(mag * mag)))
    err_abs = _CONV_EPS.get(DTYPE, 1e-3) * conv_rms
    thresh = 4.343 * err_abs / DB_ERR_TARGET
    fix = np.argwhere(mag < thresh)                        # rows: (bin, b, t)
    if len(fix):
        W64 = W.astype(np.float64)
        xp64 = x_pad.astype(np.float64)
        for b in range(B):
            sel = fix[fix[:, 1] == b]
            if not len(sel):
                continue
            for t in np.unique(sel[:, 2]):
                bins = sel[sel[:, 2] == t][:, 0]
                win = xp64[b, t * HOP:t * HOP + L_in]
                re[bins, b, t] = W64[bins] @ win
                im[bins, b, t] = W64[bins + N_BINS] @ win
        mag = np.sqrt(re * re + im * im)

    ref = max(mag.max(), AMIN)
    log_spec = 10.0 * np.log10(np.maximum(mag, AMIN)) - 10.0 * np.log10(ref)
    log_spec = np.maximum(log_spec, log_spec.max() - TOP_DB)
    return np.ascontiguousarray(log_spec.transpose(1, 2, 0)).astype(np.float32)
